# revision 1
# baseline (speedup 1.0000x reference)
"""Trainium2 Bass kernel for complex depthwise batchnorm (training-mode stats).

Data-parallel over batch N across 8 NeuronCores. Per core:
  phase A: stream the [2048, 2056] shard, accumulate per-column
           {sum xr, sum xi, sum xr^2, sum xi^2, sum xr*xi} via ones-vector
           matmuls into PSUM (fp32 matmuls for plain sums, bf16 for the
           three product sums — squares/cross are written bf16 by ACT/DVE).
  AllReduce (41KB) of the 5x2056 sums across cores.
  coefficient math on [8, 257]: 2x2 inverse-sqrt covariance whitening +
           affine mixing collapsed to y = Z@x + b' per column.
  phase B: stream the shard again, yr = Zrr*xr + Zri*xi + br',
           yi = Zir*xr + Zii*xi + bi' with coefficients broadcast across
           partitions via PE ones-broadcast; ops split across DVE + GpSimd.
"""

import numpy as np

N, C, F = 16384, 8, 257
D = C * F  # 2056
N_CORES = 8
NS = N // N_CORES  # 2048
P = 128
T = NS // P  # 16 tiles per core
EPS = 1e-6
DELTA_MAX = 1e8

# free-dim chunks for reduction matmuls (PSUM bank = 512 fp32)
# 4 full 512-wide chunks cover [0, 2048); the 8-col tail is packed separately
RED_CHUNKS = [(c * 512, 512) for c in range(4)]
TAIL_OFF, TAIL_W = 2048, D - 2048  # 8 columns
# column halves for phase B elementwise work
HALVES = [(0, D // 2), (D // 2, D - D // 2)]

_CACHE = {}


def _build():
    import concourse.bacc as bacc
    import concourse.tile as tile
    import concourse.mybir as mybir

    f32 = mybir.dt.float32
    bf16 = mybir.dt.bfloat16
    Alu = mybir.AluOpType
    Act = mybir.ActivationFunctionType

    nc = bacc.Bacc("TRN2", target_bir_lowering=False, debug=False,
                   num_devices=N_CORES)

    xr = nc.dram_tensor("xr", [NS, D], f32, kind="ExternalInput").ap()
    xi = nc.dram_tensor("xi", [NS, D], f32, kind="ExternalInput").ap()
    wrr = nc.dram_tensor("wrr", [C, F], f32, kind="ExternalInput").ap()
    wri = nc.dram_tensor("wri", [C, F], f32, kind="ExternalInput").ap()
    wii = nc.dram_tensor("wii", [C, F], f32, kind="ExternalInput").ap()
    br = nc.dram_tensor("br", [C, F], f32, kind="ExternalInput").ap()
    bi = nc.dram_tensor("bi", [C, F], f32, kind="ExternalInput").ap()
    yr = nc.dram_tensor("yr", [NS, D], f32, kind="ExternalOutput").ap()
    yi = nc.dram_tensor("yi", [NS, D], f32, kind="ExternalOutput").ap()

    with tile.TileContext(nc) as tc:
        with (
            tc.tile_pool(name="const", bufs=1) as cpool,
            tc.tile_pool(name="inp", bufs=3) as inp,
            tc.tile_pool(name="sq", bufs=8) as sqp,
            tc.tile_pool(name="tmpb", bufs=2) as tmpb,
            tc.tile_pool(name="small", bufs=1) as smp,
            tc.tile_pool(name="ctmp", bufs=6) as ctp,
            tc.tile_pool(name="dram", bufs=1, space="DRAM") as dram,
        ):
            ones_f = cpool.tile([P, 1], f32, name="ones_f")
            nc.vector.memset(ones_f[:], 1.0)
            ones_b = cpool.tile([P, 1], bf16, name="ones_b")
            nc.vector.memset(ones_b[:], 1.0)
            ones_row = cpool.tile([1, P], f32, name="ones_row")
            nc.vector.memset(ones_row[:], 1.0)

            # ---------------- phase A: local sums ----------------
            # PE matmul outputs must start at partition 0/32/64. Quantities:
            #   accA: q0=sum(xr)@p0, q1=sum(xi)@p32, q2=sum(xr^2)@p64
            #   accB: q3=sum(xi^2)@p0, q4=sum(xr*xi)@p32,
            #         all 5 tails (cols 2048:2056) @p64, free offset q*16
            cc_in = dram.tile([5, D], f32, name="cc_in")
            cc_out = dram.tile([5, D], f32, name="cc_out", addr_space="Shared")
            with tc.tile_pool(name="acc", bufs=1, space="PSUM") as accp:
                accA = accp.tile([65, 2048], f32, name="accA")  # 4 banks
                accB = accp.tile([65, 2048], f32, name="accB")  # 4 banks
                # (tile, base partition, tail free offset) per quantity
                QSLOT = [(accA, 0), (accA, 32), (accA, 64),
                         (accB, 0), (accB, 32)]

                for i in range(T):
                    xr_t = inp.tile([P, D], f32, tag="xr", name=f"xr_{i}")
                    nc.sync.dma_start(out=xr_t[:], in_=xr[i * P:(i + 1) * P, :])
                    xi_t = inp.tile([P, D], f32, tag="xi", name=f"xi_{i}")
                    nc.sync.dma_start(out=xi_t[:], in_=xi[i * P:(i + 1) * P, :])

                    st, fin = (i == 0), (i == T - 1)

                    def red(q, rhs_ap, ones_t):
                        tile_, p = QSLOT[q]
                        off, w = CUR_CHUNK
                        if off < TAIL_OFF:
                            nc.tensor.matmul(tile_[p:p + 1, off:off + w],
                                             lhsT=ones_t[:], rhs=rhs_ap,
                                             start=st, stop=fin)
                        else:
                            # all 5 tails share one 2KB zero region at
                            # accB partition 64: q0's first matmul zeroes it,
                            # q4's last matmul closes the group
                            nc.tensor.matmul(accB[64:65, q * 8:q * 8 + w],
                                             lhsT=ones_t[:], rhs=rhs_ap,
                                             start=(st and q == 0),
                                             stop=(fin and q == 4))

                    for off, w in RED_CHUNKS + [(TAIL_OFF, TAIL_W)]:
                        CUR_CHUNK = (off, w)
                        sl = slice(off, off + w)
                        red(0, xr_t[:, sl], ones_f)
                        red(1, xi_t[:, sl], ones_f)
                        sqr = sqp.tile([P, 512], bf16, tag="sqr",
                                       name=f"sqr_{i}_{off}")
                        nc.scalar.activation(sqr[:, 0:w], xr_t[:, sl],
                                             Act.Square)
                        red(2, sqr[:, 0:w], ones_b)
                        sqi = sqp.tile([P, 512], bf16, tag="sqi",
                                       name=f"sqi_{i}_{off}")
                        nc.scalar.activation(sqi[:, 0:w], xi_t[:, sl],
                                             Act.Square)
                        red(3, sqi[:, 0:w], ones_b)
                        crs = sqp.tile([P, 512], bf16, tag="crs",
                                       name=f"crs_{i}_{off}")
                        nc.vector.tensor_tensor(crs[:, 0:w], xr_t[:, sl],
                                                xi_t[:, sl], Alu.mult)
                        red(4, crs[:, 0:w], ones_b)

                # partition-aligned PSUM -> SBUF copies, then row-gather DMAs
                # (staged in the phase-B temp slots, idle at this point)
                sums_a = tmpb.tile([65, 2048], f32, tag="t1", name="sums_a")
                sums_b = tmpb.tile([65, 2048], f32, tag="t2", name="sums_b")
                nc.vector.tensor_copy(sums_a[0:1, :], accA[0:1, :])
                nc.scalar.copy(sums_a[32:33, :], accA[32:33, :])
                nc.vector.tensor_copy(sums_a[64:65, :], accA[64:65, :])
                nc.scalar.copy(sums_b[0:1, :], accB[0:1, :])
                nc.vector.tensor_copy(sums_b[32:33, :], accB[32:33, :])
                nc.scalar.copy(sums_b[64:65, 0:40], accB[64:65, 0:40])

            SB_SLOT = [(sums_a, 0), (sums_a, 32), (sums_a, 64),
                       (sums_b, 0), (sums_b, 32)]
            for q, (tile_, p) in enumerate(SB_SLOT):
                nc.sync.dma_start(out=cc_in[q:q + 1, 0:TAIL_OFF],
                                  in_=tile_[p:p + 1, :])
                nc.sync.dma_start(
                    out=cc_in[q:q + 1, TAIL_OFF:D],
                    in_=sums_b[64:65, q * 8:q * 8 + TAIL_W])

            # ---------------- all-reduce ----------------
            nc.gpsimd.collective_compute(
                "AllReduce",
                Alu.add,
                replica_groups=[list(range(N_CORES))],
                ins=[cc_in[:].opt()],
                outs=[cc_out[:].opt()],
            )
            cc_cf = cc_out[:].rearrange("q (c f) -> (q c) f", c=C)

            def load_cf(name, src):
                t = smp.tile([C, F], f32, name=name)
                nc.sync.dma_start(out=t[:], in_=src)
                return t

            s_xr = load_cf("s_xr", cc_cf[0 * C:1 * C, :])
            s_xi = load_cf("s_xi", cc_cf[1 * C:2 * C, :])
            s_rr = load_cf("s_rr", cc_cf[2 * C:3 * C, :])
            s_ii = load_cf("s_ii", cc_cf[3 * C:4 * C, :])
            s_ri = load_cf("s_ri", cc_cf[4 * C:5 * C, :])
            w_rr = load_cf("w_rr", wrr[:, :])
            w_ri = load_cf("w_ri", wri[:, :])
            w_ii = load_cf("w_ii", wii[:, :])
            b_r = load_cf("b_r", br[:, :])
            b_i = load_cf("b_i", bi[:, :])

            # ---------------- coefficient math on [C, F] ----------------
            inv_n = 1.0 / N
            V = nc.vector
            S = nc.scalar

            def keep(name):
                return smp.tile([C, F], f32, name=name)

            def scratch(name):
                return ctp.tile([C, F], f32, tag="ct", name=name)

            mr = keep("mr")
            V.tensor_scalar_mul(mr[:], s_xr[:], inv_n)
            mi = keep("mi")
            V.tensor_scalar_mul(mi[:], s_xi[:], inv_n)

            mr2 = scratch("mr2")
            V.tensor_tensor(mr2[:], mr[:], mr[:], Alu.mult)
            mi2 = scratch("mi2")
            V.tensor_tensor(mi2[:], mi[:], mi[:], Alu.mult)
            mri = scratch("mri")
            V.tensor_tensor(mri[:], mr[:], mi[:], Alu.mult)

            vrr = keep("vrr")
            V.scalar_tensor_tensor(vrr[:], s_rr[:], inv_n, mr2[:],
                                   Alu.mult, Alu.subtract)
            vii = keep("vii")
            V.scalar_tensor_tensor(vii[:], s_ii[:], inv_n, mi2[:],
                                   Alu.mult, Alu.subtract)
            vri = keep("vri")
            V.scalar_tensor_tensor(vri[:], s_ri[:], inv_n, mri[:],
                                   Alu.mult, Alu.subtract)

            tau = keep("tau")
            V.tensor_tensor(tau[:], vrr[:], vii[:], Alu.add)
            d1 = scratch("d1")
            V.tensor_tensor(d1[:], vrr[:], vii[:], Alu.mult)
            vri2 = scratch("vri2")
            V.tensor_tensor(vri2[:], vri[:], vri[:], Alu.mult)
            delta = keep("delta")
            V.tensor_tensor(delta[:], d1[:], vri2[:], Alu.subtract)
            V.tensor_scalar_max(delta[:], delta[:], EPS)
            V.tensor_scalar_min(delta[:], delta[:], DELTA_MAX)

            s_t = keep("s_t")
            S.activation(s_t[:], delta[:], Act.Sqrt)
            targ = scratch("targ")
            V.scalar_tensor_tensor(targ[:], s_t[:], 2.0, tau[:],
                                   Alu.mult, Alu.add)
            t_t = keep("t_t")
            S.activation(t_t[:], targ[:], Act.Sqrt)
            st_t = scratch("st_t")
            V.tensor_tensor(st_t[:], s_t[:], t_t[:], Alu.mult)
            rst = keep("rst")
            V.reciprocal(rst[:], st_t[:])

            a1 = scratch("a1")
            V.tensor_tensor(a1[:], s_t[:], vii[:], Alu.add)
            urr = keep("urr")
            V.tensor_tensor(urr[:], a1[:], rst[:], Alu.mult)
            a2 = scratch("a2")
            V.tensor_tensor(a2[:], s_t[:], vrr[:], Alu.add)
            uii = keep("uii")
            V.tensor_tensor(uii[:], a2[:], rst[:], Alu.mult)
            uri = keep("uri")
            V.scalar_tensor_tensor(uri[:], vri[:], -1.0, rst[:],
                                   Alu.mult, Alu.mult)

            def mix(name, wa, ua, wb, ub):
                g1 = scratch(name + "_g1")
                V.tensor_tensor(g1[:], wa[:], ua[:], Alu.mult)
                g2 = scratch(name + "_g2")
                V.tensor_tensor(g2[:], wb[:], ub[:], Alu.mult)
                z = keep(name)
                V.tensor_tensor(z[:], g1[:], g2[:], Alu.add)
                return z

            zrr = mix("zrr", w_rr, urr, w_ri, uri)
            zri = mix("zri", w_rr, uri, w_ri, uii)
            zir = mix("zir", w_ri, urr, w_ii, uri)
            zii = mix("zii", w_ri, uri, w_ii, uii)

            def bias(name, b0, za, zb):
                h1 = scratch(name + "_h1")
                V.tensor_tensor(h1[:], za[:], mr[:], Alu.mult)
                h2 = scratch(name + "_h2")
                V.tensor_tensor(h2[:], zb[:], mi[:], Alu.mult)
                h3 = scratch(name + "_h3")
                V.tensor_tensor(h3[:], h1[:], h2[:], Alu.add)
                bb = keep(name)
                V.tensor_tensor(bb[:], b0[:], h3[:], Alu.subtract)
                return bb

            brp = bias("brp", b_r, zrr, zri)
            bip = bias("bip", b_i, zir, zii)

            # ---------------- broadcast coeffs to [128, D] ----------------
            # repack each [C, F] coeff into a [1, D] partition-0 row (DMA),
            # then PE ones-broadcast (matmul rhs must sit at partition 0)
            bcs = []
            with tc.tile_pool(name="bps", bufs=4, space="PSUM") as bps:
                for k, coef in enumerate([zrr, zri, zir, zii, brp, bip]):
                    row = smp.tile([1, D], f32, tag="row", name=f"row{k}")
                    nc.sync.dma_start(out=row[0:1, :], in_=coef[:])
                    bc = cpool.tile([P, D], f32, name=f"bc{k}")
                    for off, w in RED_CHUNKS + [(TAIL_OFF, TAIL_W)]:
                        pb = bps.tile([P, 512], f32, tag="pb",
                                      name=f"pb{k}_{off}")
                        nc.tensor.matmul(pb[:, 0:w], lhsT=ones_row[:],
                                         rhs=row[0:1, off:off + w],
                                         start=True, stop=True)
                        nc.scalar.copy(bc[:, off:off + w], pb[:, 0:w])
                    bcs.append(bc)
            bzrr, bzri, bzir, bzii, bbrp, bbip = bcs

            # ---------------- phase B: apply ----------------
            for i in range(T):
                xr_t = inp.tile([P, D], f32, tag="xr", name=f"xr2_{i}")
                nc.sync.dma_start(out=xr_t[:], in_=xr[i * P:(i + 1) * P, :])
                xi_t = inp.tile([P, D], f32, tag="xi", name=f"xi2_{i}")
                nc.sync.dma_start(out=xi_t[:], in_=xi[i * P:(i + 1) * P, :])

                t1 = tmpb.tile([P, D], f32, tag="t1", name=f"t1_{i}")
                t2 = tmpb.tile([P, D], f32, tag="t2", name=f"t2_{i}")

                # full-width ops; xr_t/xi_t overwritten in place after reads.
                # 6 ops on DVE, 2 on GpSimd (POOL is ~2.4x slower per element
                # and shares SBUF ports with DVE).
                nc.vector.tensor_tensor(t1[:], xr_t[:], bzrr[:], Alu.mult)
                nc.gpsimd.tensor_tensor(t2[:], xi_t[:], bzri[:], Alu.mult)
                nc.gpsimd.tensor_tensor(xr_t[:], xr_t[:], bzir[:], Alu.mult)
                nc.vector.tensor_tensor(xi_t[:], xi_t[:], bzii[:], Alu.mult)
                # yr = t1 + t2 + brp
                nc.vector.tensor_tensor(t1[:], t1[:], t2[:], Alu.add)
                nc.vector.tensor_tensor(t1[:], t1[:], bbrp[:], Alu.add)
                # yi = xr_t + xi_t + bip
                nc.vector.tensor_tensor(xr_t[:], xr_t[:], xi_t[:], Alu.add)
                nc.vector.tensor_tensor(xr_t[:], xr_t[:], bbip[:], Alu.add)
                nc.sync.dma_start(out=yr[i * P:(i + 1) * P, :], in_=t1[:])
                nc.sync.dma_start(out=yi[i * P:(i + 1) * P, :], in_=xr_t[:])

    nc.compile()
    return nc


def get_nc():
    if "nc" not in _CACHE:
        _CACHE["nc"] = _build()
    return _CACHE["nc"]


def kernel(xr, xi, Wrr, Wri, Wii, Br, Bi):
    from concourse import bass_utils

    nc = get_nc()
    xr2 = np.ascontiguousarray(np.asarray(xr), dtype=np.float32).reshape(N, D)
    xi2 = np.ascontiguousarray(np.asarray(xi), dtype=np.float32).reshape(N, D)
    params = {
        "wrr": np.ascontiguousarray(np.asarray(Wrr), dtype=np.float32),
        "wri": np.ascontiguousarray(np.asarray(Wri), dtype=np.float32),
        "wii": np.ascontiguousarray(np.asarray(Wii), dtype=np.float32),
        "br": np.ascontiguousarray(np.asarray(Br), dtype=np.float32),
        "bi": np.ascontiguousarray(np.asarray(Bi), dtype=np.float32),
    }
    in_maps = []
    for r in range(N_CORES):
        m = {"xr": xr2[r * NS:(r + 1) * NS], "xi": xi2[r * NS:(r + 1) * NS]}
        m.update(params)
        in_maps.append(m)

    res = bass_utils.run_bass_kernel_spmd(nc, in_maps,
                                          core_ids=list(range(N_CORES)))
    yr_ = np.concatenate([res.results[r]["yr"] for r in range(N_CORES)], axis=0)
    yi_ = np.concatenate([res.results[r]["yi"] for r in range(N_CORES)], axis=0)
    return yr_.reshape(N, C, F), yi_.reshape(N, C, F)



# revision 8
# speedup vs baseline: 1.7461x; 1.7461x over previous
"""Trainium2 Bass kernel for complex depthwise batchnorm (training-mode stats).

Transposed data-parallel design, 8 NeuronCores, batch N split across cores.

Host side: each core's shard [NS, D] (NS=2048 rows, D=C*F=2056 cols) is cast
to bf16 and TRANSPOSED to [D_pad=2176, NS] so that the (c,f) axis lives on
SBUF partitions (17 chunks of 128) and the batch axis is the free dim. In
this layout the per-(c,f) stats are free-axis reductions (accum_out on
DVE/ACT/Pool ops - no PE, no PSUM), and the per-(c,f) coefficients are
per-partition scalars, so phase B uses:
  ACT  activation(Identity, scale=Zrr, bias=br')   (fused multiply-add)
  DVE  tensor_scalar with [P,1] scalar APs         (4x DVE perf mode)
  Pool scalar_tensor_tensor                        (fused mult+add)
Stats are AllReduced across cores as one [128, 85] fp32 block (5 quantities
x 17 chunk-columns). Outputs are written bf16 transposed; the host
transposes back and casts to fp32. bf16 end-to-end keeps rel err ~4e-3,
well inside the 2e-2 gate, and halves HBM traffic; inputs stay cached in
SBUF between the stats pass and the apply pass so HBM sees each element
exactly twice (one read, one write).
"""

import numpy as np
import ml_dtypes

N, C, F = 16384, 8, 257
D = C * F            # 2056
P = 128
NCH = 17             # ceil(D / 128)
DP = NCH * P         # 2176 (zero-padded tail rows)
N_CORES = 8
NS = N // N_CORES    # 2048 (free dim per core)
EPS = 1e-6
DELTA_MAX = 1e8
INV_N = 1.0 / N

# engine assignment per chunk (tuned for balance; see module docstring)
CROSS_DVE = {0, 3, 6, 9, 12, 15}       # cross-product mult on DVE, else Pool
XI2_ACT = {1, 3, 5, 7, 9, 11, 13, 15}  # sum(xi^2) on ACT, else DVE
T3_ACT = {0, 2, 4, 6, 8, 10, 12, 14, 16}   # phase-B t3 on ACT, else DVE
YI_DVE = {0, 3, 6, 9, 12, 15}          # phase-B yi-add on DVE, else Pool

_CACHE = {}


def _build():
    import concourse.bacc as bacc
    import concourse.tile as tile
    import concourse.mybir as mybir

    f32 = mybir.dt.float32
    bf16 = mybir.dt.bfloat16
    Alu = mybir.AluOpType
    Act = mybir.ActivationFunctionType

    nc = bacc.Bacc("TRN2", target_bir_lowering=False, debug=False,
                   num_devices=N_CORES)

    xrt = nc.dram_tensor("xrt", [DP, NS], bf16, kind="ExternalInput").ap()
    xit = nc.dram_tensor("xit", [DP, NS], bf16, kind="ExternalInput").ap()
    # wp columns: [0:17] Wrr, [17:34] Wri, [34:51] Wii, [51:68] Br, [68:85] Bi
    wp = nc.dram_tensor("wp", [P, 5 * NCH], f32, kind="ExternalInput").ap()
    yrt = nc.dram_tensor("yrt", [DP, NS], bf16, kind="ExternalOutput").ap()
    yit = nc.dram_tensor("yit", [DP, NS], bf16, kind="ExternalOutput").ap()

    V = None  # set below

    with tile.TileContext(nc) as tc:
        with (
            tc.tile_pool(name="keep", bufs=1) as keep,
            tc.tile_pool(name="scr", bufs=3) as scr,
            tc.tile_pool(name="tb", bufs=3) as tb,
            tc.tile_pool(name="co", bufs=6) as cop,
            tc.tile_pool(name="dram", bufs=1, space="DRAM") as dram,
        ):
            V = nc.vector
            S = nc.scalar
            G = nc.gpsimd

            wpt = keep.tile([P, 5 * NCH], f32, name="wpt")
            nc.sync.dma_start(out=wpt[:], in_=wp[:, :])

            # stats: [0:17] sum xr, [17:34] sum xi, [34:51] sum xr^2,
            #        [51:68] sum xi^2, [68:85] sum xr*xi
            st = keep.tile([P, 5 * NCH], f32, name="st")

            cc_in = dram.tile([P, 5 * NCH], f32, name="cc_in")
            cc_out = dram.tile([P, 5 * NCH], f32, name="cc_out",
                               addr_space="Shared")

            xr_c, xi_c = [], []
            for c in range(NCH):
                xt = keep.tile([P, NS], bf16, name=f"xr{c}")
                nc.sync.dma_start(out=xt[:], in_=xrt[c * P:(c + 1) * P, :])
                yt = keep.tile([P, NS], bf16, name=f"xi{c}")
                nc.sync.dma_start(out=yt[:], in_=xit[c * P:(c + 1) * P, :])
                xr_c.append(xt)
                xi_c.append(yt)

                # ---- phase A stats for this chunk ----
                # sum(xr^2): ACT square + free-axis accumulate
                sq = scr.tile([P, NS], bf16, tag="sq", name=f"sq{c}")
                S.activation(sq[:], xt[:], Act.Square,
                             accum_out=st[:, 34 + c:35 + c])
                # sum(xi^2)
                if c in XI2_ACT:
                    sq2 = scr.tile([P, NS], bf16, tag="sq", name=f"sq2{c}")
                    S.activation(sq2[:], yt[:], Act.Square,
                                 accum_out=st[:, 51 + c:52 + c])
                else:
                    sq2 = scr.tile([P, NS], bf16, tag="sq", name=f"sq2{c}")
                    V.tensor_tensor(sq2[:], yt[:], yt[:], Alu.mult)
                    V.tensor_scalar(sq2[:], sq2[:], 1.0, 0.0, Alu.mult,
                                    Alu.add, accum_out=st[:, 51 + c:52 + c])
                # sum(xr*xi)
                cr = scr.tile([P, NS], bf16, tag="sq", name=f"cr{c}")
                if c in CROSS_DVE:
                    V.tensor_tensor(cr[:], xt[:], yt[:], Alu.mult)
                else:
                    G.tensor_tensor(cr[:], xt[:], yt[:], Alu.mult)
                V.tensor_scalar(cr[:], cr[:], 1.0, 0.0, Alu.mult,
                                Alu.add, accum_out=st[:, 68 + c:69 + c])
                # plain sums: in-place x*1.0+0.0 with accumulate (no-op values)
                V.tensor_scalar(xt[:], xt[:], 1.0, 0.0, Alu.mult, Alu.add,
                                accum_out=st[:, 0 + c:1 + c])
                V.tensor_scalar(yt[:], yt[:], 1.0, 0.0, Alu.mult, Alu.add,
                                accum_out=st[:, 17 + c:18 + c])

            # ---- all-reduce the 5x17 per-partition sums ----
            nc.sync.dma_start(out=cc_in[:, :], in_=st[:])
            G.collective_compute(
                "AllReduce",
                Alu.add,
                replica_groups=[list(range(N_CORES))],
                ins=[cc_in[:].opt()],
                outs=[cc_out[:].opt()],
            )
            gt = keep.tile([P, 5 * NCH], f32, name="gt")
            nc.sync.dma_start(out=gt[:], in_=cc_out[:, :])

            # ---- coefficient math on [P, 17] fp32 slices ----
            NC_ = NCH

            def slq(t, q):
                return t[:, q * NC_:(q + 1) * NC_]

            def ktile(name):
                return keep.tile([P, NC_], f32, name=name)

            def stile(name):
                return cop.tile([P, NC_], f32, tag="co", name=name)

            mr = ktile("mr")
            V.tensor_scalar_mul(mr[:], slq(gt, 0), INV_N)
            mi = ktile("mi")
            V.tensor_scalar_mul(mi[:], slq(gt, 1), INV_N)

            mr2 = stile("mr2")
            V.tensor_tensor(mr2[:], mr[:], mr[:], Alu.mult)
            mi2 = stile("mi2")
            V.tensor_tensor(mi2[:], mi[:], mi[:], Alu.mult)
            mri = stile("mri")
            V.tensor_tensor(mri[:], mr[:], mi[:], Alu.mult)

            vrr = ktile("vrr")
            V.scalar_tensor_tensor(vrr[:], slq(gt, 2), INV_N, mr2[:],
                                   Alu.mult, Alu.subtract)
            vii = ktile("vii")
            V.scalar_tensor_tensor(vii[:], slq(gt, 3), INV_N, mi2[:],
                                   Alu.mult, Alu.subtract)
            vri = ktile("vri")
            V.scalar_tensor_tensor(vri[:], slq(gt, 4), INV_N, mri[:],
                                   Alu.mult, Alu.subtract)

            tau = stile("tau")
            V.tensor_tensor(tau[:], vrr[:], vii[:], Alu.add)
            dl = stile("dl")
            V.tensor_tensor(dl[:], vrr[:], vii[:], Alu.mult)
            vri2 = stile("vri2")
            V.tensor_tensor(vri2[:], vri[:], vri[:], Alu.mult)
            delta = stile("delta")
            V.tensor_tensor(delta[:], dl[:], vri2[:], Alu.subtract)
            V.tensor_scalar(delta[:], delta[:], EPS, DELTA_MAX,
                            Alu.max, Alu.min)

            s_t = ktile("s_t")
            S.activation(s_t[:], delta[:], Act.Sqrt)
            targ = stile("targ")
            V.scalar_tensor_tensor(targ[:], s_t[:], 2.0, tau[:],
                                   Alu.mult, Alu.add)
            t_t = stile("t_t")
            S.activation(t_t[:], targ[:], Act.Sqrt)
            stt_ = stile("stt")
            V.tensor_tensor(stt_[:], s_t[:], t_t[:], Alu.mult)
            rst = ktile("rst")
            V.reciprocal(rst[:], stt_[:])

            a1 = stile("a1")
            V.tensor_tensor(a1[:], s_t[:], vii[:], Alu.add)
            urr = ktile("urr")
            V.tensor_tensor(urr[:], a1[:], rst[:], Alu.mult)
            a2 = stile("a2")
            V.tensor_tensor(a2[:], s_t[:], vrr[:], Alu.add)
            uii = ktile("uii")
            V.tensor_tensor(uii[:], a2[:], rst[:], Alu.mult)
            uri = ktile("uri")
            V.scalar_tensor_tensor(uri[:], vri[:], -1.0, rst[:],
                                   Alu.mult, Alu.mult)

            def mix(name, wa, ua, wb, ub):
                g1 = stile(name + "g1")
                V.tensor_tensor(g1[:], wa, ua[:], Alu.mult)
                g2 = stile(name + "g2")
                V.tensor_tensor(g2[:], wb, ub[:], Alu.mult)
                z = ktile(name)
                V.tensor_tensor(z[:], g1[:], g2[:], Alu.add)
                return z

            zrr = mix("zrr", slq(wpt, 0), urr, slq(wpt, 1), uri)
            zri = mix("zri", slq(wpt, 0), uri, slq(wpt, 1), uii)
            zir = mix("zir", slq(wpt, 1), urr, slq(wpt, 2), uri)
            zii = mix("zii", slq(wpt, 1), uri, slq(wpt, 2), uii)

            def bias(name, b0, za, zb):
                h1 = stile(name + "h1")
                V.tensor_tensor(h1[:], za[:], mr[:], Alu.mult)
                h2 = stile(name + "h2")
                V.tensor_tensor(h2[:], zb[:], mi[:], Alu.mult)
                h3 = stile(name + "h3")
                V.tensor_tensor(h3[:], h1[:], h2[:], Alu.add)
                bb = ktile(name)
                V.tensor_tensor(bb[:], b0, h3[:], Alu.subtract)
                return bb

            brp = bias("brp", slq(wpt, 3), zrr, zri)
            bip = bias("bip", slq(wpt, 4), zir, zii)

            # ---- phase B: y = Z x + b', per chunk, coeffs are [P,1] APs ----
            for c in range(NCH):
                xt, yt = xr_c[c], xi_c[c]
                cs = slice(c, c + 1)
                t1 = tb.tile([P, NS], bf16, tag="t1", name=f"t1_{c}")
                S.activation(t1[:], xt[:], Act.Identity,
                             bias=brp[:, cs], scale=zrr[:, cs])
                t2 = tb.tile([P, NS], bf16, tag="t2", name=f"t2_{c}")
                V.tensor_scalar(t2[:], yt[:], zri[:, cs], None, Alu.mult)
                V.tensor_tensor(t2[:], t1[:], t2[:], Alu.add)
                nc.sync.dma_start(out=yrt[c * P:(c + 1) * P, :], in_=t2[:])
                # t3 = xi*Zii + bi' in place of xi (xi no longer needed)
                if c in T3_ACT:
                    S.activation(yt[:], yt[:], Act.Identity,
                                 bias=bip[:, cs], scale=zii[:, cs])
                else:
                    V.tensor_scalar(yt[:], yt[:], zii[:, cs], bip[:, cs],
                                    Alu.mult, Alu.add)
                # t4 = xr*Zir in place of xr, then yi = t4 + t3
                V.tensor_scalar(xt[:], xt[:], zir[:, cs], None, Alu.mult)
                if c in YI_DVE:
                    V.tensor_tensor(xt[:], xt[:], yt[:], Alu.add)
                else:
                    G.tensor_tensor(xt[:], xt[:], yt[:], Alu.add)
                nc.sync.dma_start(out=yit[c * P:(c + 1) * P, :], in_=xt[:])

    nc.compile()
    return nc


def get_nc():
    if "nc" not in _CACHE:
        _CACHE["nc"] = _build()
    return _CACHE["nc"]


def make_in_maps(xr, xi, Wrr, Wri, Wii, Br, Bi):
    bf = ml_dtypes.bfloat16
    xr2 = np.asarray(xr).reshape(N, D)
    xi2 = np.asarray(xi).reshape(N, D)
    xr_bf = xr2.astype(bf)
    xi_bf = xi2.astype(bf)

    def to_cols(a):
        v = np.zeros(DP, dtype=np.float32)
        v[:D] = np.asarray(a).reshape(D)
        return v.reshape(NCH, P).T

    wp = np.ascontiguousarray(
        np.concatenate([to_cols(Wrr), to_cols(Wri), to_cols(Wii),
                        to_cols(Br), to_cols(Bi)], axis=1),
        dtype=np.float32)

    in_maps = []
    for r in range(N_CORES):
        xrt = np.zeros((DP, NS), dtype=bf)
        xrt[:D] = xr_bf[r * NS:(r + 1) * NS].T
        xit = np.zeros((DP, NS), dtype=bf)
        xit[:D] = xi_bf[r * NS:(r + 1) * NS].T
        in_maps.append({"xrt": xrt, "xit": xit, "wp": wp})
    return in_maps


def kernel(xr, xi, Wrr, Wri, Wii, Br, Bi):
    from concourse import bass_utils

    nc = get_nc()
    in_maps = make_in_maps(xr, xi, Wrr, Wri, Wii, Br, Bi)
    res = bass_utils.run_bass_kernel_spmd(nc, in_maps,
                                          core_ids=list(range(N_CORES)))
    yr = np.concatenate(
        [np.asarray(res.results[r]["yrt"])[:D].T for r in range(N_CORES)],
        axis=0).astype(np.float32)
    yi = np.concatenate(
        [np.asarray(res.results[r]["yit"])[:D].T for r in range(N_CORES)],
        axis=0).astype(np.float32)
    return yr.reshape(N, C, F), yi.reshape(N, C, F)


# revision 10
# speedup vs baseline: 1.7559x; 1.0056x over previous
"""Trainium2 Bass kernel for complex depthwise batchnorm (training-mode stats).

Transposed data-parallel design, 8 NeuronCores, batch N split across cores.

Host side: each core's shard [NS, D] (NS=2048 rows, D=C*F=2056 cols) is cast
to bf16 and TRANSPOSED to [D_pad=2176, NS] so the (c,f) axis lives on SBUF
partitions (17 chunks of 128) and the batch axis is the free dim. Per-(c,f)
stats are free-axis reductions (accum_out / tensor_reduce on DVE, Square+
accum on ACT, products on Pool), and the per-(c,f) coefficients are
per-partition [128,1] scalars, so phase B uses DVE tensor_scalar (fast DVE
perf mode), ACT fused identity(x*scale+bias), and Pool tensor_tensor adds.

The 5x17 column stats are AllReduced in TWO halves (chunks 0..8 / 9..16) so
collective latency hides under phase A's tail and the first half's phase B.
All tensor_scalar ops write to scratch, never in place (in-place ts measured
~6x slower on HW). bf16 end-to-end keeps rel err ~3e-3 vs the 2e-2 gate.
"""

import numpy as np
import ml_dtypes

N, C, F = 16384, 8, 257
D = C * F            # 2056
P = 128
NCH = 17             # ceil(D / 128)
DP = NCH * P         # 2176 (zero-padded tail rows)
N_CORES = 8
NS = N // N_CORES    # 2048 (free dim per core)
EPS = 1e-6
DELTA_MAX = 1e8
INV_N = 1.0 / N

H1 = list(range(0, 9))       # first all-reduce half
H2 = list(range(9, NCH))     # second half
NQ = 5                       # stat quantities per chunk

# engine assignment per chunk
XI2_ACT = {1, 3, 5, 7, 9, 11, 13, 15}      # sum(xi^2) on ACT, else DVE
CROSS_POOL = {c for c in range(NCH) if c % 3 != 0}  # cross mult Pool, else DVE
T3_ACT = {0, 2, 4, 6, 8, 10, 12, 14, 16}   # phase-B t3 on ACT, else DVE
YI_DVE = {0, 3, 6, 9, 12, 15}              # phase-B yi-add on DVE, else Pool

_CACHE = {}

# stats column layout: half-1 block [0:45] (q*9 + idx), half-2 [45:85]
def st_col(q, c):
    if c < 9:
        return q * 9 + c
    return 45 + q * 8 + (c - 9)


def _build():
    import concourse.bacc as bacc
    import concourse.tile as tile
    import concourse.mybir as mybir

    f32 = mybir.dt.float32
    bf16 = mybir.dt.bfloat16
    Alu = mybir.AluOpType
    Act = mybir.ActivationFunctionType
    Ax = mybir.AxisListType

    nc = bacc.Bacc("TRN2", target_bir_lowering=False, debug=False,
                   num_devices=N_CORES)

    xrt = nc.dram_tensor("xrt", [DP, NS], bf16, kind="ExternalInput").ap()
    xit = nc.dram_tensor("xit", [DP, NS], bf16, kind="ExternalInput").ap()
    # wp columns: 5 quantities x 17 chunk-cols in st_col layout
    wp = nc.dram_tensor("wp", [P, 5 * NCH], f32, kind="ExternalInput").ap()
    yrt = nc.dram_tensor("yrt", [DP, NS], bf16, kind="ExternalOutput").ap()
    yit = nc.dram_tensor("yit", [DP, NS], bf16, kind="ExternalOutput").ap()

    with tile.TileContext(nc) as tc:
        with (
            tc.tile_pool(name="keep", bufs=1) as keep,
            tc.tile_pool(name="crp", bufs=2) as crp,
            tc.tile_pool(name="tb", bufs=2) as tb,
            tc.tile_pool(name="co", bufs=6) as cop,
            tc.tile_pool(name="dram", bufs=1, space="DRAM") as dram,
        ):
            V = nc.vector
            S = nc.scalar
            G = nc.gpsimd

            wpt = keep.tile([P, 5 * NCH], f32, name="wpt")
            nc.sync.dma_start(out=wpt[:], in_=wp[:, :])

            st = keep.tile([P, NQ * NCH], f32, name="st")
            # shared garbage-output tiles for accum ops (per-engine, WAW on
            # the same in-order engine costs nothing)
            dump_v = keep.tile([P, NS], bf16, name="dump_v")
            dump_a = keep.tile([P, NS], bf16, name="dump_a")

            cc_in1 = dram.tile([P, NQ * 9], f32, name="cc_in1")
            cc_out1 = dram.tile([P, NQ * 9], f32, name="cc_out1",
                                addr_space="Shared")
            cc_in2 = dram.tile([P, NQ * 8], f32, name="cc_in2")
            cc_out2 = dram.tile([P, NQ * 8], f32, name="cc_out2",
                                addr_space="Shared")

            xr_c, xi_c = [], []

            def phase_a_chunk(c):
                xt = keep.tile([P, NS], bf16, name=f"xr{c}")
                nc.sync.dma_start(out=xt[:], in_=xrt[c * P:(c + 1) * P, :])
                yt = keep.tile([P, NS], bf16, name=f"xi{c}")
                nc.sync.dma_start(out=yt[:], in_=xit[c * P:(c + 1) * P, :])
                xr_c.append(xt)
                xi_c.append(yt)

                # sum(xr^2): ACT square + accumulate
                S.activation(dump_a[:], xt[:], Act.Square,
                             accum_out=st[:, st_col(2, c):st_col(2, c) + 1])
                # sum(xi^2)
                if c in XI2_ACT:
                    S.activation(dump_a[:], yt[:], Act.Square,
                                 accum_out=st[:, st_col(3, c):st_col(3, c) + 1])
                else:
                    V.tensor_tensor(dump_v[:], yt[:], yt[:], Alu.mult)
                    V.tensor_scalar(dump_v[:], dump_v[:], 1.0, 0.0, Alu.mult,
                                    Alu.add,
                                    accum_out=st[:, st_col(3, c):st_col(3, c) + 1])
                # sum(xr*xi)
                if c in CROSS_POOL:
                    cr = crp.tile([P, NS], bf16, tag="cr", name=f"cr{c}")
                    G.tensor_tensor(cr[:], xt[:], yt[:], Alu.mult)
                    V.tensor_scalar(dump_v[:], cr[:], 1.0, 0.0, Alu.mult,
                                    Alu.add,
                                    accum_out=st[:, st_col(4, c):st_col(4, c) + 1])
                else:
                    V.tensor_tensor(dump_v[:], xt[:], yt[:], Alu.mult)
                    V.tensor_scalar(dump_v[:], dump_v[:], 1.0, 0.0, Alu.mult,
                                    Alu.add,
                                    accum_out=st[:, st_col(4, c):st_col(4, c) + 1])
                # plain sums: A/B two reduce flavors to compare on HW
                V.tensor_scalar(dump_v[:], xt[:], 1.0, 0.0, Alu.mult, Alu.add,
                                accum_out=st[:, st_col(0, c):st_col(0, c) + 1])
                V.tensor_reduce(st[:, st_col(1, c):st_col(1, c) + 1], yt[:],
                                Ax.X, Alu.add)

            # coefficient tiles, one column per chunk
            zrr = keep.tile([P, NCH], f32, name="zrr")
            zri = keep.tile([P, NCH], f32, name="zri")
            zir = keep.tile([P, NCH], f32, name="zir")
            zii = keep.tile([P, NCH], f32, name="zii")
            brp = keep.tile([P, NCH], f32, name="brp")
            bip = keep.tile([P, NCH], f32, name="bip")

            def coeff_math(h, gt, w):
                """gt: all-reduced [P, 5*nc_] sums (q-major); w: same layout
                params; writes coeff columns [P, lo:hi]."""
                nc_ = len(h)
                lo, hi = h[0], h[-1] + 1
                cs = slice(lo, hi)

                def q(t, i):
                    return t[:, i * nc_:(i + 1) * nc_]

                def stile(name):
                    # [P, <=9] fp32 tiles are 36B/partition: keep them all
                    return keep.tile([P, nc_], f32, name=f"{name}_{lo}")

                mr = stile("mr")
                V.tensor_scalar_mul(mr[:], q(gt, 0), INV_N)
                mi = stile("mi")
                V.tensor_scalar_mul(mi[:], q(gt, 1), INV_N)

                mr2 = stile("mr2")
                V.tensor_tensor(mr2[:], mr[:], mr[:], Alu.mult)
                mi2 = stile("mi2")
                V.tensor_tensor(mi2[:], mi[:], mi[:], Alu.mult)
                mri = stile("mri")
                V.tensor_tensor(mri[:], mr[:], mi[:], Alu.mult)

                vrr = stile("vrr")
                V.scalar_tensor_tensor(vrr[:], q(gt, 2), INV_N, mr2[:],
                                       Alu.mult, Alu.subtract)
                vii = stile("vii")
                V.scalar_tensor_tensor(vii[:], q(gt, 3), INV_N, mi2[:],
                                       Alu.mult, Alu.subtract)
                vri = stile("vri")
                V.scalar_tensor_tensor(vri[:], q(gt, 4), INV_N, mri[:],
                                       Alu.mult, Alu.subtract)

                tau = stile("tau")
                V.tensor_tensor(tau[:], vrr[:], vii[:], Alu.add)
                dl = stile("dl")
                V.tensor_tensor(dl[:], vrr[:], vii[:], Alu.mult)
                vri2 = stile("vri2")
                V.tensor_tensor(vri2[:], vri[:], vri[:], Alu.mult)
                delta = stile("delta")
                V.tensor_tensor(delta[:], dl[:], vri2[:], Alu.subtract)
                V.tensor_scalar(delta[:], delta[:], EPS, DELTA_MAX,
                                Alu.max, Alu.min)

                s_t = stile("s_t")
                S.activation(s_t[:], delta[:], Act.Sqrt)
                targ = stile("targ")
                V.scalar_tensor_tensor(targ[:], s_t[:], 2.0, tau[:],
                                       Alu.mult, Alu.add)
                t_t = stile("t_t")
                S.activation(t_t[:], targ[:], Act.Sqrt)
                stt_ = stile("stt")
                V.tensor_tensor(stt_[:], s_t[:], t_t[:], Alu.mult)
                rst = stile("rst")
                V.reciprocal(rst[:], stt_[:])

                a1 = stile("a1")
                V.tensor_tensor(a1[:], s_t[:], vii[:], Alu.add)
                urr = stile("urr")
                V.tensor_tensor(urr[:], a1[:], rst[:], Alu.mult)
                a2 = stile("a2")
                V.tensor_tensor(a2[:], s_t[:], vrr[:], Alu.add)
                uii = stile("uii")
                V.tensor_tensor(uii[:], a2[:], rst[:], Alu.mult)
                uri = stile("uri")
                V.scalar_tensor_tensor(uri[:], vri[:], -1.0, rst[:],
                                       Alu.mult, Alu.mult)

                def mix(zt, wa, ua, wb, ub, nm):
                    g1 = stile(nm + "g1")
                    V.tensor_tensor(g1[:], wa, ua[:], Alu.mult)
                    g2 = stile(nm + "g2")
                    V.tensor_tensor(g2[:], wb, ub[:], Alu.mult)
                    V.tensor_tensor(zt[:, cs], g1[:], g2[:], Alu.add)

                mix(zrr, q(w, 0), urr, q(w, 1), uri, "zrr")
                mix(zri, q(w, 0), uri, q(w, 1), uii, "zri")
                mix(zir, q(w, 1), urr, q(w, 2), uri, "zir")
                mix(zii, q(w, 1), uri, q(w, 2), uii, "zii")

                def bias(bt, b0, za, zb, nm):
                    h1 = stile(nm + "h1")
                    V.tensor_tensor(h1[:], za[:, cs], mr[:], Alu.mult)
                    h2 = stile(nm + "h2")
                    V.tensor_tensor(h2[:], zb[:, cs], mi[:], Alu.mult)
                    h3 = stile(nm + "h3")
                    V.tensor_tensor(h3[:], h1[:], h2[:], Alu.add)
                    V.tensor_tensor(bt[:, cs], b0, h3[:], Alu.subtract)

                bias(brp, q(w, 3), zrr, zri, "brp")
                bias(bip, q(w, 4), zir, zii, "bip")

            def phase_b_chunk(c):
                xt, yt = xr_c[c], xi_c[c]
                cs = slice(c, c + 1)
                # t1 = xr*Zrr + br'
                t1 = tb.tile([P, NS], bf16, tag="t1", name=f"t1_{c}")
                if c in T3_ACT:
                    V.tensor_scalar(t1[:], xt[:], zrr[:, cs], brp[:, cs],
                                    Alu.mult, Alu.add)
                else:
                    S.activation(t1[:], xt[:], Act.Identity,
                                 bias=brp[:, cs], scale=zrr[:, cs])
                # t2 = xi*Zri ; yr = t1 + t2 (in-place tt into t2 is fine)
                t2 = tb.tile([P, NS], bf16, tag="t2", name=f"t2_{c}")
                V.tensor_scalar(t2[:], yt[:], zri[:, cs], None, Alu.mult)
                V.tensor_tensor(t2[:], t1[:], t2[:], Alu.add)
                nc.sync.dma_start(out=yrt[c * P:(c + 1) * P, :], in_=t2[:])
                # t3 = xi*Zii + bi'
                t3 = tb.tile([P, NS], bf16, tag="t3", name=f"t3_{c}")
                if c in T3_ACT:
                    S.activation(t3[:], yt[:], Act.Identity,
                                 bias=bip[:, cs], scale=zii[:, cs])
                else:
                    V.tensor_scalar(t3[:], yt[:], zii[:, cs], bip[:, cs],
                                    Alu.mult, Alu.add)
                # t4 = xr*Zir ; yi = t4 + t3 (in-place tt into t4)
                t4 = tb.tile([P, NS], bf16, tag="t4", name=f"t4_{c}")
                V.tensor_scalar(t4[:], xt[:], zir[:, cs], None, Alu.mult)
                if c in YI_DVE:
                    V.tensor_tensor(t4[:], t4[:], t3[:], Alu.add)
                else:
                    G.tensor_tensor(t4[:], t4[:], t3[:], Alu.add)
                nc.sync.dma_start(out=yit[c * P:(c + 1) * P, :], in_=t4[:])

            # ---------------- schedule ----------------
            for c in H1:
                phase_a_chunk(c)
            nc.sync.dma_start(out=cc_in1[:, :], in_=st[:, 0:NQ * 9])
            G.collective_compute(
                "AllReduce", Alu.add,
                replica_groups=[list(range(N_CORES))],
                ins=[cc_in1[:].opt()], outs=[cc_out1[:].opt()])

            for c in H2:
                phase_a_chunk(c)
            nc.sync.dma_start(out=cc_in2[:, :], in_=st[:, NQ * 9:])
            G.collective_compute(
                "AllReduce", Alu.add,
                replica_groups=[list(range(N_CORES))],
                ins=[cc_in2[:].opt()], outs=[cc_out2[:].opt()])

            gt1 = keep.tile([P, NQ * 9], f32, name="gt1")
            nc.sync.dma_start(out=gt1[:], in_=cc_out1[:, :])
            coeff_math(H1, gt1, wpt[:, 0:NQ * 9])
            for c in H1:
                phase_b_chunk(c)

            gt2 = keep.tile([P, NQ * 8], f32, name="gt2")
            nc.sync.dma_start(out=gt2[:], in_=cc_out2[:, :])
            coeff_math(H2, gt2, wpt[:, NQ * 9:])
            for c in H2:
                phase_b_chunk(c)

    nc.compile()
    return nc


def get_nc():
    if "nc" not in _CACHE:
        _CACHE["nc"] = _build()
    return _CACHE["nc"]


def make_in_maps(xr, xi, Wrr, Wri, Wii, Br, Bi):
    bf = ml_dtypes.bfloat16
    xr2 = np.asarray(xr).reshape(N, D)
    xi2 = np.asarray(xi).reshape(N, D)
    xr_bf = xr2.astype(bf)
    xi_bf = xi2.astype(bf)

    # params -> [P, 5*NCH] in the st_col layout (q-major inside each half)
    def to_cols(a):
        v = np.zeros(DP, dtype=np.float32)
        v[:D] = np.asarray(a).reshape(D)
        return v.reshape(NCH, P).T          # [P, NCH], col c = chunk c

    cols = [to_cols(Wrr), to_cols(Wri), to_cols(Wii), to_cols(Br), to_cols(Bi)]
    wp = np.zeros((P, 5 * NCH), dtype=np.float32)
    for q in range(5):
        wp[:, q * 9:(q + 1) * 9] = cols[q][:, 0:9]
        wp[:, 45 + q * 8:45 + (q + 1) * 8] = cols[q][:, 9:17]

    in_maps = []
    for r in range(N_CORES):
        xrt = np.zeros((DP, NS), dtype=bf)
        xrt[:D] = xr_bf[r * NS:(r + 1) * NS].T
        xit = np.zeros((DP, NS), dtype=bf)
        xit[:D] = xi_bf[r * NS:(r + 1) * NS].T
        in_maps.append({"xrt": xrt, "xit": xit, "wp": wp})
    return in_maps


def kernel(xr, xi, Wrr, Wri, Wii, Br, Bi):
    from concourse import bass_utils

    nc = get_nc()
    in_maps = make_in_maps(xr, xi, Wrr, Wri, Wii, Br, Bi)
    res = bass_utils.run_bass_kernel_spmd(nc, in_maps,
                                          core_ids=list(range(N_CORES)))
    yr = np.concatenate(
        [np.asarray(res.results[r]["yrt"])[:D].T for r in range(N_CORES)],
        axis=0).astype(np.float32)
    yi = np.concatenate(
        [np.asarray(res.results[r]["yit"])[:D].T for r in range(N_CORES)],
        axis=0).astype(np.float32)
    return yr.reshape(N, C, F), yi.reshape(N, C, F)
